# revision 49
# baseline (speedup 1.0000x reference)
"""BERT self-attention (B=4, S=2048, H=768, 12 heads x 64) on 8 trn2 cores.

Sharding: core c = batch (c//2) x head-half (c%2, 6 heads each).
Each core computes Q/K/V projections for its 6 heads, attention, and a
partial output projection (its heads' slice of Wo). Host sums the two
partials per batch and adds bo.

v3 = baseline + fp8-DoubleRow attn@V. The PE on trn2 drains fp32 to
PSUM at 1 col/cycle, so matmul cost is N/2.4 ns regardless of input
dtype; DoubleRow (fp8, K=256 per instruction) halves the attn@V
accumulation steps: 8 key-tile-pair MMs per (hp, sq, head) instead of
16. V is computed in bf16 (6-chunk projection, as baseline) and STORED
fp8 in pair-tiles; exp writes fp8 directly. Everything else (QKV/out
projections, scores) stays bf16 — fp8 there fails the accuracy budget
(out-proj cancellation amplifies quantization ~2e-2 alone).

Scheduling: attn@V(pair p) is emitted one slot AFTER exp(2p+1), i.e.
at slot 2p+2, so the pair MMs (which wait on that exp) never sit at
the PE queue head blocking the next slot's score matmuls — that
head-of-line block made exp wait on scores transitively and triggered
HAM K=4/8 oscillation (half-rate PE ~48% of the time). Each group's
pair-7 attn@V + normalize consequently run in the next group's slot
0-1 (or the tail for the last group).

Output streams as st-PAIR DMAs (3KB/partition contiguous runs on both
sides — the DRAM out tensor is partition-major [128, 16, 768] and the
host transposes back); the last two pairs go on different queues so
their transfers overlap the tail.

Emission order is a correctness contract: Tile builds WAR deps from
emission order, so every tile's writers must be emitted before its
readers (v_pending / inject gates), and cb tiles are allocated at
kt==2, after the previous group's normalize (their last reader) has
been emitted.

Measured dead ends (do not revisit): fp8 for QKV/out projections
fails accuracy (out-proj cancellation amplifies quantization to
~2.5e-2 alone); a 2048-wide exp needs 6 score banks + 2 cb + >=1
projection bank > 8 PSUM banks; offloading some exp tiles to the DVE
(one-op Schraudolph into fp8 bits — works numerically, +1.4e-3) put a
fourth engine at >50% busy and tripped a chip-level power clamp that
downclocked everything ~0.85x, a 50us net LOSS; PE warm-up dummy
matmuls either delay the real chains (FIFO) or re-cool in the gap
before the DMAs land.
"""

import numpy as np
import ml_dtypes

B, S, H = 4, 2048, 768
NH, HS = 12, 64
NHL = 6              # heads per core
NHP = 3              # head pairs per core
HCHUNKS = 6          # 768 / 128 contraction chunks
SKT = 16             # key tiles of 128
NPAIR = 8            # key-tile pairs
SQT = 4              # query tiles of 512
QW = 512             # query tile width
VW = 400             # v2 plane width (6*65 padded to %16)
N_CORES = 8

_COMPILED = None


def _build():
    import concourse.bass as bass
    import concourse.mybir as mybir
    import concourse.tile as tile
    from concourse import bacc

    fp32 = mybir.dt.float32
    bf16 = mybir.dt.bfloat16
    f8 = mybir.dt.float8e4
    u8 = mybir.dt.uint8
    AF = mybir.ActivationFunctionType
    ALU = mybir.AluOpType
    DR = mybir.MatmulPerfMode.DoubleRow
    # DVE one-op fast-exp: uint8 bits = round(s*log2e*8*0.125 + B) are
    # fp8e4m3 of ~exp(s/8) (Schraudolph in 8 bits; DVE cast rounds).
    # Offload these key tiles' exp from ACT (the pacing engine) to DVE.
    FE_A = 1.4426950408889634
    FE_B = 55.55
    FE_KT = ()

    nc = bacc.Bacc("TRN2", target_bir_lowering=False, debug=False)

    xtp_d = [nc.dram_tensor(f"xt{p}", [128, HCHUNKS, QW], bf16,
                            kind="ExternalInput").ap() for p in range(SQT)]
    wq0_d = nc.dram_tensor("wq0", [128, HCHUNKS, 128], bf16,
                           kind="ExternalInput").ap()
    wk0_d = nc.dram_tensor("wk0", [128, HCHUNKS, 128], bf16,
                           kind="ExternalInput").ap()
    wqr_d = nc.dram_tensor("wqr", [128, HCHUNKS, 256], bf16,
                           kind="ExternalInput").ap()
    wkr_d = nc.dram_tensor("wkr", [128, HCHUNKS, 256], bf16,
                           kind="ExternalInput").ap()
    wv_d = nc.dram_tensor("wv", [128, HCHUNKS, NHL * HS], bf16,
                          kind="ExternalInput").ap()
    wo_d = nc.dram_tensor("wo", [128, NHP, H], bf16,
                          kind="ExternalInput").ap()
    b6_d = nc.dram_tensor("b6", [6, 134], bf16, kind="ExternalInput").ap()
    bvr_d = nc.dram_tensor("bvr", [1, NHL * HS], bf16,
                           kind="ExternalInput").ap()
    mask_d = nc.dram_tensor("mask", [128, SKT], fp32, kind="ExternalInput").ap()
    # partition-major output: [p, st, c] so st-pair stream-outs are
    # contiguous 3KB runs on both sides (host transposes back)
    out_d = nc.dram_tensor("out", [128, S // 128, H], bf16,
                           kind="ExternalOutput").ap()

    with tile.TileContext(nc) as tc:
        with (
            tc.tile_pool(name="const", bufs=1) as const,
            tc.tile_pool(name="xt", bufs=1) as xtp,
            tc.tile_pool(name="vsb", bufs=1) as vsb,
            tc.tile_pool(name="qkt", bufs=2) as qkt,
            tc.tile_pool(name="combt", bufs=1) as combtp,
            tc.tile_pool(name="oacc", bufs=1) as oaccp,
            tc.tile_pool(name="attn", bufs=3) as attnp,
            tc.tile_pool(name="small", bufs=4) as smallp,
            tc.tile_pool(name="ps_sc", bufs=2, space="PSUM") as ps_sc,
            tc.tile_pool(name="ps_cb", bufs=2, space="PSUM") as ps_cb,
            tc.tile_pool(name="ps_pj", bufs=2, space="PSUM") as ps_pj,
        ):
            xt_t = [xtp.tile([128, HCHUNKS, QW], bf16, tag=f"xt{p}",
                             name=f"xt{p}") for p in range(SQT)]
            wk0_t = const.tile([128, HCHUNKS, 128], bf16, tag="wk0")
            wq0_t = const.tile([128, HCHUNKS, 128], bf16, tag="wq0")
            wkr_t = const.tile([128, HCHUNKS, 256], bf16, tag="wkr")
            wqr_t = const.tile([128, HCHUNKS, 256], bf16, tag="wqr")
            wv_t = const.tile([128, HCHUNKS, NHL * HS], bf16, tag="wv")
            b6_sb = const.tile([6, 134], bf16, tag="b6")
            bvr_sb = const.tile([1, NHL * HS], bf16, tag="bvr")
            bias_cols = const.tile([128, 2 * NHP], fp32, tag="bcols")
            bv_sb = const.tile([128, NHL * HS], fp32, tag="bv")
            mask_sb = const.tile([128, SKT], fp32, tag="mask")
            mask2_sb = const.tile([128, SKT], fp32, tag="mask2")
            wo_t = const.tile([128, NHP, H], bf16, tag="wo")
            ones_sb = const.tile([1, 256], bf16, tag="ones")
            nc.vector.memset(ones_sb[:], 1.0)

            # startup DMAs: one sync FIFO in consumption order; b6/bvr
            # first (tiny, and the bias-spread matmuls sit at the head
            # of the PE queue — a late b6 head-of-line-blocks the chains)
            nc.sync.dma_start(b6_sb[:], b6_d)
            nc.sync.dma_start(bvr_sb[:], bvr_d)
            nc.sync.dma_start(xt_t[0][:], xtp_d[0])
            nc.sync.dma_start(wk0_t[:], wk0_d)
            nc.sync.dma_start(wq0_t[:], wq0_d)
            nc.sync.dma_start(mask_sb[:], mask_d[:])
            nc.sync.dma_start(wv_t[:], wv_d)
            nc.sync.dma_start(xt_t[1][:], xtp_d[1])
            nc.sync.dma_start(wkr_t[:], wkr_d)
            nc.sync.dma_start(wqr_t[:], wqr_d)
            nc.sync.dma_start(xt_t[2][:], xtp_d[2])
            nc.sync.dma_start(xt_t[3][:], xtp_d[3])
            nc.sync.dma_start(wo_t[:], wo_d)
            # spread the biases with tiny bf16 PE matmuls
            ps_b = ps_pj.tile([128, 512], fp32, tag="pj", name="psb")
            nc.tensor.matmul(ps_b[:, 0:6], lhsT=b6_sb[0:6, 0:128],
                             rhs=b6_sb[0:6, 128:134], start=True, stop=True)
            nc.vector.tensor_copy(bias_cols[:], ps_b[:, 0:6])
            ps_b2 = ps_pj.tile([128, 512], fp32, tag="pj", name="psb2")
            nc.tensor.matmul(ps_b2[:, 0:NHL * HS], lhsT=ones_sb[0:1, 0:128],
                             rhs=bvr_sb[:], start=True, stop=True)
            nc.vector.tensor_copy(bv_sb[:], ps_b2[:, 0:NHL * HS])
            # fast-exp bias column: mask*8*log2e + B (per key partition)
            nc.vector.tensor_scalar(mask2_sb[:], mask_sb[:], 8.0 * FE_A,
                                    FE_B, ALU.mult, ALU.add)

            # ---- V projection (bf16, 6 chunks) into fp8 pair-tiles:
            # v2[p][:, j, h*65+d], col h*65+64 = 1.0 (denominator lands
            # on cb row 64). Emitted as per-kt unit chains. ----
            v_sb = [vsb.tile([128, 2, VW], f8, tag=f"v{p}", name=f"v{p}")
                    for p in range(NPAIR)]

            def v_units(kt):
                vt = v_sb[kt // 2]
                jj = kt % 2
                st8 = {}

                def unit(c, st8=st8):
                    if c == 0:
                        st8["ps"] = ps_pj.tile([128, 512], fp32, tag="pj",
                                               name="psv")
                    nc.tensor.matmul(
                        st8["ps"][:, :NHL * HS],
                        lhsT=xt_t[kt // 4][:, c,
                                           (kt % 4) * 128:(kt % 4 + 1) * 128],
                        rhs=wv_t[:, c, :],
                        start=(c == 0),
                        stop=(c == HCHUNKS - 1),
                    )
                    if c == HCHUNKS - 1:
                        vview = vt[:, jj, 0:NHL * 65].rearrange(
                            "p (h d) -> p h d", h=NHL, d=65)
                        nc.vector.tensor_add(
                            vview[:, :, 0:HS],
                            st8["ps"][:, :NHL * HS].rearrange(
                                "p (h d) -> p h d", h=NHL),
                            bv_sb[:].rearrange("p (h d) -> p h d", h=NHL),
                        )
                        nc.vector.memset(vview[:, :, 64:65], 1.0)

                return [lambda c=c: unit(c) for c in range(HCHUNKS)]

            combt = [combtp.tile([128, S], bf16, tag=f"ct{hp}", name=f"ct{hp}")
                     for hp in range(NHP)]
            out_acc = oaccp.tile([128, S // 128, H], bf16, tag="oacc")

            def emit_qkt(hp):
                qt_t = qkt.tile([128, S], bf16, tag="qt", name=f"qt{hp}")
                kt_t = qkt.tile([128, S], bf16, tag="kt", name=f"kt{hp}")
                units = []
                chains = {}
                for kind, dst, w_t, bcol in (
                        ("kt", kt_t, wk0_t if hp == 0 else wkr_t, 0),
                        ("qt", qt_t, wq0_t if hp == 0 else wqr_t, NHP)):
                    for sq in range(SQT):
                        st8 = {}

                        def unit(c, dst=dst, w_t=w_t, bcol=bcol, sq=sq, st8=st8):
                            if c == 0:
                                st8["ps"] = ps_pj.tile(
                                    [128, 512], fp32, tag="pj", name="psq")
                            nc.tensor.matmul(
                                st8["ps"][:],
                                lhsT=(w_t[:, c, :] if hp == 0 else
                                      w_t[:, c, (hp - 1) * 128:hp * 128]),
                                rhs=xt_t[sq][:, c, :],
                                start=(c == 0),
                                stop=(c == HCHUNKS - 1),
                            )
                            if c == HCHUNKS - 1:
                                nc.vector.tensor_scalar_add(
                                    dst[:, sq * QW:(sq + 1) * QW], st8["ps"][:],
                                    bias_cols[:, bcol + hp:bcol + hp + 1],
                                )

                        chain = [lambda c=c, u=unit: u(c)
                                 for c in range(HCHUNKS)]
                        chains[(kind, sq)] = chain
                        units.extend(chain)
                return qt_t, kt_t, units, chains

            def emit_outproj_unit(hp, st, half, stream_out=False,
                                  out_q=None):
                ps = ps_pj.tile([128, 512], fp32, tag="pj", name="pso")
                nc.tensor.matmul(
                    ps[:, 0:384],
                    lhsT=combt[hp][:, st * 128:(st + 1) * 128],
                    rhs=wo_t[:, hp, half * 384:(half + 1) * 384],
                    start=True, stop=True,
                )
                dst = out_acc[:, st, half * 384:(half + 1) * 384]
                if hp == 0:
                    nc.vector.tensor_copy(dst, ps[:, 0:384])
                else:
                    nc.vector.tensor_add(dst, dst, ps[:, 0:384])
                if stream_out:
                    # st-pair DMA: contiguous 3KB runs on both sides
                    (out_q or nc.sync).dma_start(
                        out_d[:, st - 1:st + 1, :],
                        out_acc[:, st - 1:st + 1, :])

            def outproj_units(hp, sqs, stream_out=False, min_sq=None):
                units = []
                for sq in sqs:
                    gate = sq + 2 if min_sq is None else min_sq
                    for st in range(4 * sq, 4 * (sq + 1)):
                        for half in range(2):
                            units.append((gate, lambda hp=hp, st=st, half=half,
                                          so=(stream_out and half == 1
                                              and st % 2 == 1):
                                          emit_outproj_unit(hp, st, half, so)))
                return units

            slots = [(hp, sq, kt) for hp in range(NHP) for sq in range(SQT)
                     for kt in range(SKT)]

            def scores(hp, sq, kt):
                qt_t, kt_t = qkts[hp][0], qkts[hp][1]
                sc = ps_sc.tile([128, 1024], fp32, tag="sc", name="sc")
                nc.tensor.matmul(
                    sc[:, 0:512],
                    lhsT=kt_t[0:64, kt * 128:(kt + 1) * 128],
                    rhs=qt_t[0:64, sq * QW:(sq + 1) * QW],
                    start=True, stop=True,
                )
                nc.tensor.matmul(
                    sc[:, 512:1024],
                    lhsT=kt_t[64:128, kt * 128:(kt + 1) * 128],
                    rhs=qt_t[64:128, sq * QW:(sq + 1) * QW],
                    start=True, stop=True,
                )
                return sc

            def emit_attnv(hp, p, at_t, cb_pair, stop):
                """DR attn@V for key-tile pair p of head pair hp."""
                cb_a, cb_b = cb_pair
                vt = v_sb[p]
                nc.tensor.matmul(
                    cb_a[:],
                    lhsT=vt[:, :, 2 * hp * 65:(2 * hp + 1) * 65],
                    rhs=at_t[:, :, 0:512],
                    start=(p == 0), stop=stop,
                    perf_mode=DR,
                )
                nc.tensor.matmul(
                    cb_b[:],
                    lhsT=vt[:, :, (2 * hp + 1) * 65:(2 * hp + 2) * 65],
                    rhs=at_t[:, :, 512:1024],
                    start=(p == 0), stop=stop,
                    perf_mode=DR,
                )

            def emit_normalize(hp, sq, cb_pair, last_group):
                """comb rows 0..63 / denom (row 64) -> combt[hp]."""
                cbs_list = []
                for cb in cb_pair:
                    cbs = smallp.tile([65, 512], fp32, tag="cbs", name="cbs")
                    if last_group:
                        nc.scalar.copy(cbs[:], cb[:])
                    else:
                        nc.vector.tensor_copy(cbs[:], cb[:])
                    cbs_list.append(cbs)
                for half, cbs in ((0, cbs_list[0]), (1, cbs_list[1])):
                    rc0 = smallp.tile([1, 512], fp32, tag="rc0")
                    nc.sync.dma_start(rc0[:], cbs[64:65, :])
                    if last_group:
                        rc1 = smallp.tile([1, 512], fp32, tag="rc1")
                        nc.vector.reciprocal_approx_fast(rc1[:], rc0[:])
                        rc1b = smallp.tile([1, 512], bf16, tag="rc1b")
                        nc.scalar.copy(rc1b[:], rc1[:])
                        bc_ps = ps_pj.tile([128, 512], fp32, tag="pj",
                                           name="bcps")
                        nc.tensor.matmul(bc_ps[0:64, :],
                                         lhsT=ones_sb[0:1, 0:64],
                                         rhs=rc1b[:], start=True, stop=True)
                        bc_ap = bc_ps[0:64, :]
                    else:
                        rc1 = smallp.tile([1, 512], fp32, tag="rc1")
                        nc.vector.reciprocal_approx_fast(rc1[:], rc0[:])
                        bc = smallp.tile([64, 512], fp32, tag="bc")
                        nc.gpsimd.partition_broadcast(bc[:], rc1[:])
                        bc_ap = bc[:]
                    nc.vector.tensor_mul(
                        combt[hp][64 * half:64 * (half + 1),
                                  sq * QW:(sq + 1) * QW],
                        cbs[0:64, :], bc_ap,
                    )

            # ---- pre-loop ----
            qkts = [emit_qkt(0)]
            ch0 = qkts[0][3]
            for u in ch0[("kt", 0)] + ch0[("qt", 0)]:
                u()
            sc_cur = scores(*slots[0])
            v_pending = {kt: v_units(kt) for kt in range(SKT)}

            inject_q = {0: [], 1: [], 2: []}
            qkts.append(emit_qkt(1))
            # kt chains cover KEY regions: all needed during sq0
            # (scores(sq0, kt=4r) reads the ("kt", r) chain's columns)
            inject_q[0] = (
                [(0, u) for u in ch0[("kt", 1)]]
                + [(0, u) for u in ch0[("kt", 2)]]
                + [(0, u) for u in ch0[("kt", 3)]]
                + [(0, u) for u in ch0[("qt", 1)]]
                + [(1, u) for u in ch0[("qt", 2)]]
                + [(2, u) for u in ch0[("qt", 3)]]
                # hp1's chains gated to sq2+: sq0/sq1 run PE-oversubscribed
                # (v chains + JIT kt/qt chains) and starve ACT ~1.2us every
                # ~4 slots; sq2/sq3 have ~8us of exp-wait slack each. The
                # last unit pops at sq3 slot ~11, before the hp1 lookahead.
                + [(2, u) for u in qkts[1][2]]
                + outproj_units(0, range(SQT - 2)))

            at_cur = None
            at_prev = None      # completed at pair awaiting its attn@V
            cb_cur = None       # current group's cb pair (alloc at kt==2)
            prev_group = None   # (hp, sq) of the group cb_cur belongs to
            for i, (hp, sq, kt) in enumerate(slots):
                if kt == 0 and sq == 0 and hp > 0:
                    for _, u in inject_q[hp - 1]:
                        u()
                    inject_q[hp - 1] = []
                if kt == 0 and sq == 0 and hp == 1:
                    qkts.append(emit_qkt(2))
                    inject_q[1] = (outproj_units(0, [SQT - 2, SQT - 1],
                                                 min_sq=0)
                                   + [(0, u) for u in qkts[2][2]]
                                   + outproj_units(1, range(SQT - 2)))
                if kt == 0 and sq == 0 and hp == 2:
                    inject_q[2] = (outproj_units(1, [SQT - 2, SQT - 1],
                                                 min_sq=0)
                                   + outproj_units(2, range(SQT - 2),
                                                   stream_out=True)
                                   + outproj_units(2, [SQT - 2],
                                                   stream_out=True,
                                                   min_sq=SQT - 1))
                # lookahead scores for the next slot (keeps PE fed and
                # ensures exp(i+1) is never blocked behind attn@V)
                sc_nxt = scores(*slots[i + 1]) if i + 1 < len(slots) else None
                if kt % 2 == 0:
                    at_cur = attnp.tile([128, 2, 1024], f8, tag="at")
                if kt in FE_KT:
                    # offloaded exp: DVE writes fp8 bits directly
                    nc.vector.tensor_scalar(
                        at_cur[:, kt % 2, :].bitcast(u8), sc_cur[:],
                        FE_A, mask2_sb[:, kt:kt + 1], ALU.mult, ALU.add)
                else:
                    nc.scalar.activation(
                        at_cur[:, kt % 2, :], sc_cur[:], AF.Exp,
                        bias=mask_sb[:, kt:kt + 1], scale=0.125,
                    )
                # deferred attn@V: pair (kt//2 - 1), or the previous
                # group's pair 7 (+ normalize) at kt==0
                if kt % 2 == 0:
                    if kt == 0:
                        if cb_cur is not None:
                            emit_attnv(prev_group[0], NPAIR - 1, at_prev,
                                       cb_cur, stop=True)
                            emit_normalize(prev_group[0], prev_group[1],
                                           cb_cur, last_group=False)
                        prev_group = (hp, sq)
                    elif kt == 2:
                        cb_a = ps_cb.tile([65, 512], fp32, tag="cb",
                                          name="cba")
                        cb_b = ps_cb.tile([65, 512], fp32, tag="cb",
                                          name="cbb")
                        cb_cur = (cb_a, cb_b)
                        emit_attnv(hp, 0, at_prev, cb_cur, stop=False)
                    else:
                        emit_attnv(hp, kt // 2 - 1, at_prev, cb_cur,
                                   stop=False)
                if hp == 0 and sq == 0:
                    for k in (kt, kt + 1):
                        if k in v_pending:
                            for u in v_pending.pop(k):
                                u()
                # fill the PE exp-wait bubble with independent work;
                # keep boundary slots quiet so the group handoff
                # (scores/attnV/normalize) isn't delayed behind fillers
                q = inject_q[hp]
                popped = 0
                max_pop = 2 if hp == 0 else (3 if (hp == 1 and sq == 0) else 2)
                j = 0
                while j < len(q) and popped < max_pop:
                    if q[j][0] <= sq:
                        q.pop(j)[1]()
                        popped += 1
                    else:
                        j += 1
                if kt % 2 == 1:
                    at_prev = at_cur
                sc_cur = sc_nxt

            # ---- tail: last group's pair 7 + normalize, leftovers ----
            emit_attnv(NHP - 1, NPAIR - 1, at_prev, cb_cur, stop=True)
            emit_normalize(NHP - 1, SQT - 1, cb_cur, last_group=True)
            for hp in range(NHP):
                for _, u in inject_q[hp]:
                    u()
                inject_q[hp] = []
            # tail: (12,13) pair on the scalar queue; st14/st15 as
            # singles on sync so st14's transfer starts as soon as its
            # add lands and the final exposed transfer is one st deep
            for st in range(4 * (SQT - 1), 4 * SQT):
                for half in range(2):
                    emit_outproj_unit(
                        2, st, half,
                        stream_out=(half == 1 and st == 13),
                        out_q=nc.scalar)
                if st >= 14:
                    nc.sync.dma_start(out_d[:, st:st + 1, :],
                                      out_acc[:, st:st + 1, :])

    nc.compile()
    return nc


def _get_compiled():
    global _COMPILED
    if _COMPILED is None:
        _COMPILED = _build()
    return _COMPILED


def _b6(bkc, bqc):
    """[6, 134] block: rows 0-2 bk per head pair, 3-5 bq; cols 128-133
    a 6x6 identity used to transpose the rows into [128, 6] on-device."""
    b6 = np.zeros((6, 134), dtype=ml_dtypes.bfloat16)
    b6[0:3, 0:128] = bkc.reshape(NHP, 128)
    b6[3:6, 0:128] = bqc.reshape(NHP, 128)
    b6[:, 128:134] = np.eye(6)
    return b6


def _prep_core_inputs(x, mask, Wq, bq, Wk, bk, Wv, bv, Wo, core):
    b, hg = core // 2, core % 2
    lo, hi = hg * NHL * HS, (hg + 1) * NHL * HS
    bf = ml_dtypes.bfloat16

    def chunked(w):  # [768, D] -> [128, HCHUNKS, D]
        return np.ascontiguousarray(
            w.reshape(HCHUNKS, 128, -1).transpose(1, 0, 2)).astype(bf)

    xt = x[b].T.reshape(HCHUNKS, 128, S)  # [c][p][s]
    im = {
        "wq0": chunked(Wq[:, lo:lo + 128]),
        "wk0": chunked(Wk[:, lo:lo + 128]),
        "wqr": chunked(Wq[:, lo + 128:hi]),
        "wkr": chunked(Wk[:, lo + 128:hi]),
        "wv": chunked(Wv[:, lo:hi]),
        "wo": np.ascontiguousarray(
            Wo[lo:hi, :].reshape(NHP, 128, H).transpose(1, 0, 2)).astype(bf),
        "b6": _b6(bk[lo:hi], bq[lo:hi]),
        "bvr": bv[lo:hi][None, :].astype(bf),
        "mask": np.ascontiguousarray(
            mask[b, 0, 0].reshape(SKT, 128).T).astype(np.float32),
    }
    for p in range(SQT):
        im[f"xt{p}"] = np.ascontiguousarray(
            xt[:, :, p * QW:(p + 1) * QW].transpose(1, 0, 2)).astype(bf)
    return im


def kernel(x, additive_attention_mask, Wq, bq, Wk, bk, Wv, bv, Wo, bo):
    from concourse import bass2jax

    x = np.asarray(x, dtype=np.float32)
    mask = np.asarray(additive_attention_mask, dtype=np.float32)
    args = [np.asarray(a, dtype=np.float32) for a in (Wq, bq, Wk, bk, Wv, bv, Wo)]
    Wq, bq, Wk, bk, Wv, bv, Wo = args
    bo = np.asarray(bo, dtype=np.float32)

    nc = _get_compiled()
    in_maps = [
        _prep_core_inputs(x, mask, Wq, bq, Wk, bk, Wv, bv, Wo, c)
        for c in range(N_CORES)
    ]
    results = bass2jax.run_bass_via_pjrt(nc, in_maps, n_cores=N_CORES)

    out = np.empty((B, S, H), dtype=np.float32)
    for b in range(B):
        # device layout [p, st, c] -> [S, H]
        o0 = results[2 * b]["out"].astype(np.float32)
        o1 = results[2 * b + 1]["out"].astype(np.float32)
        out[b] = (o0 + o1).transpose(1, 0, 2).reshape(S, H) + bo
    return out
